# revision 63
# baseline (speedup 1.0000x reference)
# Trainium2 Bass kernel for the ASE (axial squeeze attention) block.
#
# Sharding: pure data parallel over batch B=16 across 8 NeuronCores
# (2 batches per core); all params replicated.
#
# v3 restructure (94.1us modeled vs the 140.6us baseline):
#  - per-batch work split into stage-A (means-H accumulation + q/k convs +
#    gate evictions -- attention-independent) and stage-B (v conv, xx path,
#    pointwise + final convs). Stage-A/attention of batch 1 and the v/u
#    pre-runs of batch 0 fill batch 0's attention-latency window.
#  - both batches' attentions run CONCURRENTLY: the 6 psum-ring tags
#    (qk2, v2, o, xp, att, mh) are time-shared by phase -- attn0 d0/d1 on
#    att/xp, attn1 d0/d1 on mh/o, means-W on att(b0)/o(b1), stage-B gated
#    outputs on o+mh (b0) and the qk pair (b1), all xp on att, v-conv
#    psums on a 3-bank ring (attn0-d1 shares it ahead of them). Ring-slot
#    grant order always matches emission order to avoid scheduler slot
#    deadlocks.
#  - transposed softmax denominator: per-head ones-matmuls -> [nseq, 8]
#    reciprocal; normalization folded into the xrT eviction broadcast-mult
#    (removes the rb-matmul + eTn pass of the baseline).
#  - means-H accumulates per-chunk (4-row sub-blocks so both k-tiles share
#    one bank) and pipelines with the x DMA; means-W packs both k-tiles in
#    one bank via t=4 sub-groups.
#  - v conv evicted ONCE as u = v + bv (bf16); gate z_v and the xx path both
#    read u, so the v psum frees without waiting on attention.
#  - eviction placement by measured cost model rates (ACT 611/psum-op, DVE
#    ts 658 psum / 194 bf16-sbuf 4x, Pool 2-input 0.42 eff): z_q on DVE,
#    z_k/u on ACT, z_v cheap-ts on DVE (batch1 m1 on ACT), rc merged
#    [128,2,8,64] on Pool, r6_m1 on Pool, rest on DVE. Placements were
#    tuned against the TimelineSim schedule, not just demand totals.
#  - x / q,k,v weights in bf16 (same PE rate as f32r, half DMA+SBUF);
#    pointwise conv in bf16; fp8 DoubleRow final conv rescaled weights x16
#    with xx/16 on the DVE evict so the ACT r-eviction needs no scale.
#  - identity for the means matmuls built on-chip (memset + affine_select).
#  - single ordered DMA queue: x0, critical weights (incl. the attention
#    projection slice of wbigb), x1, remaining weights; y stored in
#    2-chunk [128, 1024] tiles.
import numpy as np

import concourse.bass as bass
import concourse.mybir as mybir
import concourse.tile as tile
from concourse import bacc, bass_utils

F32 = mybir.dt.float32
F32R = mybir.dt.float32r
BF16 = mybir.dt.bfloat16
FP8 = mybir.dt.float8e4
AF = mybir.ActivationFunctionType
ALU = mybir.AluOpType
AX = mybir.AxisListType
DR = mybir.MatmulPerfMode.DoubleRow

B, DIM, H, W = 16, 256, 64, 64
KEY_DIM, HEADS = 16, 8
NH_KD = KEY_DIM * HEADS   # 128
DH = 2 * KEY_DIM * HEADS  # 256
POS = 16
N_CORES = 8
BPC = B // N_CORES        # batches per core

MMDT = F32R  # dtype of x / q,k,v weights feeding the PE

WQKV_COLS = 128 * 2 + 128 * 2 + 256 * 2          # q,k,v (f32r)
WPW_COLS = 256 * 4                                # pw (bf16, 4 k-tiles)
WBIGB_COLS = 512 + 512 + 512 + 512 + 512 + 64    # wr,wc,wq,wk,wv,id64


def build_nc(bpc=BPC, h=H, w=W, chunk_h=8, num_devices=N_CORES,
             use_f32r=False, nrep=1, tiny_out=False):
    """Build the per-core Bass module."""
    global MMDT
    MMDT = F32R if use_f32r else BF16
    hw = h * w
    n_chunks = h // chunk_h
    nc_cols = chunk_h * w      # spatial columns per chunk

    nc = bacc.Bacc("TRN2", target_bir_lowering=False, debug=False,
                   num_devices=num_devices)

    dram = {}

    def din(name, shape, dt=None):
        dram[name] = nc.dram_tensor(name, shape, dt or F32,
                                    kind="ExternalInput").ap()
        return dram[name]

    din("x", (bpc, DIM, hw), MMDT)
    din("wqkv", (128, WQKV_COLS), MMDT)
    din("wpwb", (128, WPW_COLS), BF16)
    din("wbigb", (128, WBIGB_COLS), BF16)
    din("wp8", (128, 512), FP8)
    din("qkbias", (128, 512))
    din("params", (128, 20))
    din("onesW", (max(h, w), 1), BF16)
    din("ones1", (1, 64), BF16)
    y_cols = nc_cols if tiny_out else hw
    y_d = nc.dram_tensor("y", (bpc, DIM, y_cols), BF16,
                         kind="ExternalOutput").ap()

    with tile.TileContext(nc) as tc:
        _emit(nc, tc, dram, y_d, bpc, h, w, hw, chunk_h, n_chunks, nc_cols,
              nrep, tiny_out)
    nc.compile()
    return nc


def _emit(nc, tc, dram, y_d, bpc, h, w, hw, chunk_h, n_chunks, nc_cols,
          nrep=1, tiny_out=False):
    from contextlib import ExitStack
    with ExitStack() as _ctx:
        _emit_body(_ctx, nc, tc, dram, y_d, bpc, h, w, hw, chunk_h, n_chunks,
                   nc_cols, nrep, tiny_out)


def _emit_body(ctx, nc, tc, dram, y_d, bpc, h, w, hw, chunk_h, n_chunks,
               nc_cols, nrep=1, tiny_out=False):
    ctx.enter_context(nc.allow_low_precision(
        reason="bf16/fp8 matmul operand rounding"))
    # ---- persistent weights / params ----
    wp = ctx.enter_context(tc.tile_pool(name="weights", bufs=1))

    # identity built on-chip (saves the startup DMA round-trip)
    ident128 = wp.tile([128, 128], MMDT, tag="ident", name="ident128")
    ones128 = wp.tile([128, 128], MMDT, tag="ones128", name="ones128")
    nc.gpsimd.memset(ones128, 1.0)
    nc.gpsimd.affine_select(out=ident128, in_=ones128, pattern=[[1, 128]],
                            compare_op=ALU.is_equal, fill=0.0,
                            base=0, channel_multiplier=-1)
    wqkv = wp.tile([128, WQKV_COLS], MMDT, tag="wqkv")
    qkb = wp.tile([128, 512], F32, tag="qkb")
    wbigb = wp.tile([128, WBIGB_COLS], BF16, tag="wbigb")
    params = wp.tile([128, 20], F32, tag="params")
    onesW = wp.tile([max(h, w), 1], BF16, tag="onesW")   # value = W
    ones1 = wp.tile([1, 64], BF16, tag="ones1")
    wpwb = wp.tile([128, WPW_COLS], BF16, tag="wpwb")
    wp8t = [wp.tile([128, 2, 128], FP8, tag=f"wp8_{mo}", name=f"wp8_{mo}")
            for mo in range(2)]

    def load_weights_crit():
        # ordered behind batch-0's x on the same queue: conv weights + the
        # small attention consts
        nc.sync.dma_start(out=wqkv, in_=dram["wqkv"])
        nc.sync.dma_start(out=qkb, in_=dram["qkbias"])
        nc.sync.dma_start(out=params, in_=dram["params"])
        nc.sync.dma_start(out=onesW, in_=dram["onesW"])
        nc.sync.dma_start(out=ones1, in_=dram["ones1"])
        # attention q/k/v projection weights only (cols 1024:2560); the
        # wr/wc conv weights follow after x1
        nc.sync.dma_start(out=wbigb[:, 1024:2560],
                          in_=dram["wbigb"][:, 1024:2560])

    def load_weights_rest():
        nc.sync.dma_start(out=wbigb[:, 0:1024], in_=dram["wbigb"][:, 0:1024])
        nc.sync.dma_start(out=wbigb[:, 2560:WBIGB_COLS],
                          in_=dram["wbigb"][:, 2560:WBIGB_COLS])
        nc.sync.dma_start(out=wpwb, in_=dram["wpwb"])
        # DoubleRow stationary tiles must be native [p, 2, m] 3-D tiles
        for mo in range(2):
            nc.sync.dma_start(
                out=wp8t[mo].rearrange("p i f -> p (i f)"),
                in_=dram["wp8"][:, 256 * mo:256 * (mo + 1)])

    def _slices(tile_, widths):
        out, off = [], 0
        for wd in widths:
            out.append(tile_[:, off:off + wd])
            off += wd
        return out

    (wqT0, wqT1, wkT0, wkT1, wvT0, wvT1) = _slices(
        wqkv, [NH_KD, NH_KD, NH_KD, NH_KD, DH, DH])
    wqT, wkT, wvT = [wqT0, wqT1], [wkT0, wkT1], [wvT0, wvT1]
    wpwT = _slices(wpwb, [256, 256, 256, 256])   # per input k-tile, [m0|m1]
    (wrT0, wrT1, wcT0, wcT1, wqTp0, wqTp1, wkTp0, wkTp1, wvTb0, wvTb1,
     ident64b) = _slices(
        wbigb, [DH, DH, DH, DH, 256, 256, 256, 256, DH, DH, 64])
    wrT, wcT = [wrT0, wrT1], [wcT0, wcT1]
    wqTp, wkTp, wvTb = [wqTp0, wqTp1], [wkTp0, wkTp1], [wvTb0, wvTb1]
    ident64 = ident64b[:64, :]

    # param columns
    zbias = [params[:, 2 + g:3 + g] for g in range(4)]    # q, k, v0, v1
    brv = [params[:, 8 + m:9 + m] for m in range(2)]
    bcc = [params[:, 10 + m:11 + m] for m in range(2)]
    bp3 = [params[:, 12 + m:13 + m] for m in range(2)]
    bpw6 = [params[:, 14 + m:15 + m] for m in range(2)]

    # ---- pools ----
    px = ctx.enter_context(tc.tile_pool(name="x", bufs=2))
    pa = ctx.enter_context(tc.tile_pool(name="attn", bufs=1))
    pz = ctx.enter_context(tc.tile_pool(name="z", bufs=2))
    pc = ctx.enter_context(tc.tile_pool(name="chunk", bufs=4))
    pout = ctx.enter_context(tc.tile_pool(name="outb", bufs=2))
    pp = ctx.enter_context(tc.tile_pool(name="psum", bufs=1, space="PSUM"))

    def phase_load(b):
        xs = [px.tile([128, hw], MMDT, tag=f"xs{k}", name=f"xs{b}_{k}")
              for k in range(2)]
        for gs in (slice(0, 1024), slice(1024, 2048), slice(2048, hw)):
            for k in range(2):
                nc.sync.dma_start(out=xs[k][:, gs],
                                  in_=dram["x"][b, 128 * k:128 * (k + 1), gs])
        return xs

    def xm_alloc():
        # [128, 128] bf16: cols 0..63 = W-dir sums (per h), 64..127 = H-dir
        # sums (per w)
        return [pa.tile([128, 128], BF16, tag=f"xm{k}", bufs=2,
                        name=f"xm{k}") for k in range(2)]

    def stage_a(b, xs, mh_ps, xm, c):
        """Per-chunk: means-H accumulation + q/k convs + gate evictions."""
        cs = slice(c * nc_cols, (c + 1) * nc_cols)
        # means-H: 2 sub-blocks of 4 rows x 2 k-tiles into one bank
        with tc.high_priority(offset=400):
            for k in range(2):
                for s in range(2):
                    ss = slice(c * nc_cols + s * 256,
                               c * nc_cols + (s + 1) * 256)
                    nc.tensor.matmul(mh_ps[:, 256 * k:256 * k + 256],
                                     lhsT=ident128, rhs=xs[k][:, ss],
                                     start=(c == 0 and s == 0),
                                     stop=(c == n_chunks - 1 and s == 1))
        # q/k convs (weights carry the depthwise gate scale)
        zqk = []
        for gi, wT in enumerate((wqT, wkT)):
            ps = pp.tile([128, nc_cols], F32, tag="qk", bufs=2,
                         name=f"qk{b}{c}{gi}")
            for k in range(2):
                nc.tensor.matmul(ps, lhsT=wT[k], rhs=xs[k][:, cs],
                                 start=(k == 0), stop=(k == 1))
            sb = pz.tile([128, nc_cols], BF16, tag=f"z{gi}", bufs=16)
            nc.scalar.activation(out=sb, in_=ps, func=AF.Relu,
                                 bias=zbias[gi])
            zqk.append(sb)
        return zqk

    def means_h_reduce(mh_ps, xm):
        for k in range(2):
            with tc.high_priority(offset=64):
                nc.vector.tensor_reduce(
                    out=xm[k][:, 64:64 + w].unsqueeze(-1),
                    in_=mh_ps[:, 256 * k:256 * (k + 1)].rearrange(
                        "p (s w) -> p w s", s=4),
                    axis=AX.X, op=ALU.add)

    def means_w(b, xs, xm, mw_tag="att"):
        mw_ps = pp.tile([128, 512], F32, tag=mw_tag, name=f"mw{b}")
        for k in range(2):
            xv = xs[k].rearrange("p (h j t) -> p j h t", j=16, t=4)
            for j in range(16):
                nc.tensor.matmul(mw_ps[:, 256 * k:256 * (k + 1)],
                                 lhsT=ident128, rhs=xv[:, j],
                                 start=(j == 0), stop=(j == 15))
        for k in range(2):
            with tc.high_priority(offset=64):
                nc.vector.tensor_reduce(
                    out=xm[k][:, 0:h].unsqueeze(-1),
                    in_=mw_ps[:, 256 * k:256 * (k + 1)].rearrange(
                        "p (h t) -> p h t", t=4),
                    axis=AX.X, op=ALU.add)

    def phase_attn(b, xm, d0_tags=("att",), d1_tags=("xp",)):
        """Both axial attentions. Returns (xr_t, xc_t): [128, 2, 64] bf16
        conv outputs + bias, m-merged for the Pool broadcast. The d=1 chain
        can run on a different psum ring (d1_tag) so both directions
        overlap."""
        qk_ps = pp.tile([128, 512], F32, tag=d0_tags[0], name=f"qk_ps{b}")
        for wi, wT in enumerate((wqTp, wkTp)):
            for t in range(2):
                sl = slice(256 * wi + 128 * t, 256 * wi + 128 * (t + 1))
                for k in range(2):
                    nc.tensor.matmul(qk_ps[:, sl],
                                     lhsT=wT[k][:, 128 * t:128 * (t + 1)],
                                     rhs=xm[k], start=(k == 0), stop=(k == 1))
        # evict per-direction halves so the d=1 chain needn't wait for d=0's
        qk_sb = pa.tile([128, 512], BF16, tag="qk_sb", bufs=2)
        qk_v = qk_sb.rearrange("p (blk d i) -> p d blk i", blk=4, d=2)
        qkp_v = qk_ps.rearrange("p (blk d i) -> p d blk i", blk=4, d=2)
        qkb_v = qkb.rearrange("p (blk d i) -> p d blk i", blk=4, d=2)
        for d in range(2):
            nc.vector.tensor_tensor(out=qk_v[:, d], in0=qkp_v[:, d],
                                    in1=qkb_v[:, d], op=ALU.add)

        xatt = [None, None]
        for d in range(2):          # 0 = row (nseq=h), 1 = col (nseq=w)
            ptags = list(d0_tags) if d == 0 else list(d1_tags)
            _ti = [0]

            def ptile(shape, dt, name):
                tg = ptags[_ti[0] % len(ptags)]
                _ti[0] += 1
                return pp.tile(shape, dt, tag=tg,
                               bufs=(2 if tg == "qk" else (3 if tg == "v" else 1)),
                               name=name)
            nseq = h if d == 0 else w
            do = 64 * d
            eT = pa.tile([nseq, HEADS * nseq], BF16, tag=f"at_e{d}", bufs=2)
            eTv = eT.rearrange("p (t j i) -> p j t i", t=2, j=4)
            for j in range(4):
                st_ps = ptile([nseq, 2 * nseq], F32, f"st_ps{b}{d}{j}")
                for t in range(2):
                    ksl = qk_sb[32 * j:32 * (j + 1),
                                256 + 128 * t + do:256 + 128 * t + do + 64]
                    qsl = qk_sb[32 * j:32 * (j + 1),
                                128 * t + do:128 * t + do + 64]
                    nc.tensor.matmul(st_ps[:, nseq * t:nseq * (t + 1)],
                                     lhsT=ksl, rhs=qsl, start=True, stop=True,
                                     tile_position=(32 * j, 0))
                nc.scalar.activation(
                    out=eTv[:, j], in_=st_ps.rearrange("p (t i) -> p t i", t=2),
                    func=AF.Exp, scale=KEY_DIM ** -0.5)
            # transposed softmax denominator: srowT[i, h] = W * sum_j e
            # (onesW value = W folds the mean), reciprocal on [nseq, 8],
            # normalization folded into the xrT eviction as a broadcast mult.
            srowT_ps = ptile([nseq, HEADS], F32, f"srT{b}{d}")
            for hh in range(HEADS):
                nc.tensor.matmul(srowT_ps[:, hh:hh + 1],
                                 lhsT=eT[:, nseq * hh:nseq * (hh + 1)],
                                 rhs=onesW[:nseq, :], start=True, stop=True)
            recipT = pa.tile([nseq, HEADS], BF16, tag=f"at_rc{d}", bufs=2)
            nc.vector.reciprocal(out=recipT, in_=srowT_ps)
            vrT_ps = ptile([nseq, DH], F32, f"vrT{b}{d}")
            for k in range(2):
                nc.tensor.matmul(vrT_ps, lhsT=xm[k][:, do:do + nseq],
                                 rhs=wvTb[k], start=(k == 0), stop=(k == 1))
            vrT = pa.tile([nseq, DH], BF16, tag=f"at_vs{d}", bufs=2)
            nc.vector.tensor_copy(out=vrT, in_=vrT_ps)
            xrT_ps = ptile([nseq, DH], F32, f"xrT{b}{d}")
            for hh in range(HEADS):
                nc.tensor.matmul(xrT_ps[:, 32 * hh:32 * (hh + 1)],
                                 lhsT=eT[:, nseq * hh:nseq * (hh + 1)],
                                 rhs=vrT[:, 32 * hh:32 * (hh + 1)],
                                 start=True, stop=True)
            xrT_sb = pa.tile([nseq, DH], BF16, tag=f"at_xt{d}", bufs=2)
            nc.vector.tensor_tensor(
                out=xrT_sb.rearrange("p (h e) -> p h e", h=HEADS),
                in0=xrT_ps.rearrange("p (h e) -> p h e", h=HEADS),
                in1=recipT.unsqueeze(-1).broadcast_to((nseq, HEADS, 32)),
                op=ALU.mult)
            # transpose back to (channel, i), relu(+bv folded in params) on
            # eviction; then conv (dh -> dh) + bias into merged [128,2,64]
            xr_relu = []
            for t in range(2):
                tr_ps = ptile([128, nseq], BF16, f"at_tr{b}{d}_{t}")
                nc.tensor.transpose(tr_ps, xrT_sb[:, 128 * t:128 * (t + 1)],
                                    ident64[:nseq, :nseq])
                sb = pa.tile([128, nseq], BF16, tag=f"at_xrr{d}_{t}", bufs=2)
                nc.scalar.activation(out=sb, in_=tr_ps, func=AF.Relu,
                                     bias=params[:, 6 + t:7 + t])
                xr_relu.append(sb)
            wconvT = wrT if d == 0 else wcT
            bconv = brv if d == 0 else bcc
            xa_t = pa.tile([128, 2, nseq], BF16, tag=f"at_xa{d}", bufs=2)
            for m in range(2):
                ps = ptile([128, nseq], F32, f"at_cv{b}{d}{m}")
                for k in range(2):
                    nc.tensor.matmul(ps,
                                     lhsT=wconvT[k][:, 128 * m:128 * (m + 1)],
                                     rhs=xr_relu[k],
                                     start=(k == 0), stop=(k == 1))
                nc.scalar.activation(out=xa_t[:, m], in_=ps, func=AF.Identity,
                                     bias=bconv[m])
            xatt[d] = xa_t
        return xatt[0], xatt[1]

    def stage_b(b, xs, zqk_list, xr_t, xc_t, c):
        cs = slice(c * nc_cols, (c + 1) * nc_cols)
        hs = slice(c * chunk_h, (c + 1) * chunk_h)
        # v conv, evicted ONCE as u = v + bv (feeds both gate and xx paths;
        # frees the v psum without waiting on attention)
        ut = pz.tile([128, 2, nc_cols], BF16, tag="u", bufs=6,
                     name=f"u{b}{c}")
        for mo in range(2):
            ps = pp.tile([128, nc_cols], F32, tag="v", bufs=3,
                         name=f"v{b}{c}{mo}")
            for k in range(2):
                nc.tensor.matmul(ps, lhsT=wvT[k][:, 128 * mo:128 * (mo + 1)],
                                 rhs=xs[k][:, cs], start=(k == 0), stop=(k == 1))
            nc.scalar.activation(out=ut[:, mo], in_=ps, func=AF.Identity,
                                 bias=params[:, 6 + mo:7 + mo])
        us = [ut[:, 0], ut[:, 1]]
        # gate z_v = relu(u + bdw/g) on DVE 4x (g folded into pw weights)
        zv = []
        for mo in range(2):
            sb = pz.tile([128, nc_cols], BF16, tag=f"zv{mo}", bufs=6)
            nc.vector.tensor_scalar(out=sb, in0=us[mo], scalar1=zbias[2 + mo],
                                    scalar2=0.0, op0=ALU.add, op1=ALU.max)
            zv.append(sb)
        # rc = xr[h] + xc[w], both m halves in one Pool op
        rc = pc.tile([128, 2, chunk_h, w], BF16, tag="rc")
        nc.gpsimd.tensor_tensor(
            out=rc,
            in0=xr_t[:, :, hs].unsqueeze(-1).broadcast_to(
                (128, 2, chunk_h, w)),
            in1=xc_t.unsqueeze(2).broadcast_to((128, 2, chunk_h, w)),
            op=ALU.add)
        # xx = relu(u + rc)/16 -> fp8, both m halves per op
        xx8 = pc.tile([128, 2, 512], FP8, tag="xx8")
        t = pc.tile([128, 2, nc_cols], BF16, tag="xx")
        with tc.high_priority(offset=64):
            nc.vector.tensor_tensor(
                out=t, in0=ut,
                in1=rc.rearrange("p m h w -> p m (h w)"),
                op=ALU.add)
            nc.vector.tensor_scalar(
                out=xx8, in0=t.rearrange("p m n -> p (m n)").rearrange(
                    "p (m n) -> p m n", m=2),
                scalar1=0.0, scalar2=1.0 / 16.0,
                op0=ALU.max, op1=ALU.mult)
        # per output half: pointwise conv (bf16) -> final conv (fp8 DR,
        # weights x16) -> r -> r6 -> gated output. Emission order matches
        # the o/xp ring rotation so slot grants can't cycle.
        z = zqk_list + zv   # [z_q, z_k, z_v0, z_v1]
        # final-conv chain first so r6 is ready when the pointwise psum
        # needs gating (shortens the o-bank hold to pw->stt)
        r6s = []
        for m in range(2):
            xp_ps = pp.tile([128, nc_cols], F32, tag="att",
                            name=f"xp{b}{c}{m}")
            nc.tensor.matmul(xp_ps, lhsT=wp8t[m], rhs=xx8,
                             start=True, stop=True, perf_mode=DR)
            r = pc.tile([128, nc_cols], BF16, tag=f"r{m}")
            nc.scalar.activation(out=r, in_=xp_ps, func=AF.Relu, bias=bp3[m])
            r6 = pc.tile([128, nc_cols], BF16, tag=f"r6{m}")
            eng = nc.vector if m == 0 else nc.gpsimd
            eng.tensor_scalar(out=r6, in0=r, scalar1=6.0,
                              scalar2=0.0, op0=ALU.min, op1=ALU.add)
            r6s.append(r6)
        for m in range(2):
            # m=0 on the "o" bank, m=1 reuses the (long-idle) "mh" bank so
            # the two gated-output chains don't serialize on one psum slot.
            o_tag = ("o" if m == 0 else "mh") if b == 0 else "qk"
            qkv_ps = pp.tile([128, nc_cols], F32, tag=o_tag,
                             bufs=(2 if o_tag == "qk" else 1),
                             name=f"o{b}{c}{m}")
            for k in range(4):
                nc.tensor.matmul(
                    qkv_ps, lhsT=wpwT[k][:, 128 * m:128 * (m + 1)],
                    rhs=z[k], start=(k == 0), stop=(k == 3))
            r6 = r6s[m]
            if tiny_out:
                o = pout.tile([128, nc_cols], BF16, tag=f"ob{m}", bufs=3)
                nc.vector.scalar_tensor_tensor(
                    out=o, in0=qkv_ps, scalar=bpw6[m], in1=r6,
                    op0=ALU.add, op1=ALU.mult)
                nc.sync.dma_start(
                    out=y_d[b, 128 * m:128 * (m + 1), 0:nc_cols], in_=o)
            else:
                o2 = ob_cur[m]
                nc.vector.scalar_tensor_tensor(
                    out=o2[:, (c % 2) * nc_cols:(c % 2 + 1) * nc_cols],
                    in0=qkv_ps, scalar=bpw6[m], in1=r6,
                    op0=ALU.add, op1=ALU.mult)
                if c % 2 == 1:
                    nc.sync.dma_start(
                        out=y_d[b, 128 * m:128 * (m + 1),
                                (c - 1) * nc_cols:(c + 1) * nc_cols],
                        in_=o2)

    for _ in range(nrep):
        # batch 0: load + stage-A per chunk (pipelines with the DMA)
        xs0 = phase_load(0)
        load_weights_crit()
        xs1 = phase_load(1)
        load_weights_rest()
        xm0 = xm_alloc()
        mh0 = pp.tile([128, 512], F32, tag="mh", name="mh0")
        zqk0 = [stage_a(0, xs0, mh0, xm0, c) for c in range(n_chunks)]
        means_h_reduce(mh0, xm0)
        with tc.high_priority(offset=3000):
            means_w(0, xs0, xm0)
            at0 = phase_attn(0, xm0, d0_tags=("att",), d1_tags=("v",))
        # batch 1 stage-A fills batch-0 attention bubbles
        xm1 = xm_alloc()
        mh1 = pp.tile([128, 512], F32, tag="mh", name="mh1")
        zqk1 = [stage_a(1, xs1, mh1, xm1, c) for c in range(n_chunks)]
        means_h_reduce(mh1, xm1)
        # batch 0 stage-B leads; means-W1 + attn1 between; then interleave
        lead = 5
        ob_map = {}

        def run_b(b, c):
            nonlocal ob_cur
            if c % 2 == 0:
                ob_map[b] = [pout.tile([128, 2 * nc_cols], BF16,
                                       tag=f"ob{m}", bufs=3,
                                       name=f"ob{b}{c}{m}")
                             for m in range(2)]
            ob_cur = ob_map[b]
            zq = zqk0[c] if b == 0 else zqk1[c]
            at = at0 if b == 0 else at1
            xs = xs0 if b == 0 else xs1
            stage_b(b, xs, zq, *at, c)

        ob_cur = None
        with tc.high_priority(offset=3000):
            means_w(1, xs1, xm1, mw_tag="o")
            at1 = phase_attn(1, xm1, d0_tags=("o",), d1_tags=("mh",))
        for c in range(lead):
            run_b(0, c)
        units = [(0, c) for c in range(lead, n_chunks)]
        units1 = [(1, c) for c in range(n_chunks)]
        order = []
        while units or units1:
            if units:
                order.append(units.pop(0))
            for _ in range(2):
                if units1:
                    order.append(units1.pop(0))
        for b, c in order:
            run_b(b, c)


# ---------------------------------------------------------------------------
# host-side preparation
# ---------------------------------------------------------------------------

def _interp_pos_np(pe, n):
    s = pe.shape[-1]
    pos = np.clip((np.arange(n, dtype=np.float64) + 0.5) * (s / n) - 0.5,
                  0.0, s - 1.0).astype(np.float32)
    i0 = np.floor(pos).astype(np.int32)
    i1 = np.minimum(i0 + 1, s - 1)
    fw = (pos - i0).astype(np.float32)
    return pe[:, i0] * (1.0 - fw) + pe[:, i1] * fw


def prepare_consts(inputs, h=H, w=W, chunk_h=8):
    """Fold BN/scales and build the constant tensors the kernel expects."""
    import ml_dtypes
    f = lambda a: np.ascontiguousarray(np.asarray(a, dtype=np.float32))
    fb = lambda a: np.ascontiguousarray(
        np.asarray(a, dtype=np.float32).astype(ml_dtypes.bfloat16))
    Wq, sq, bq = f(inputs["Wq"]), f(inputs["sq"]), f(inputs["bq"])
    Wk, sk, bk = f(inputs["Wk"]), f(inputs["sk"]), f(inputs["bk"])
    Wv, sv, bv = f(inputs["Wv"]), f(inputs["sv"]), f(inputs["bv"])
    wdw, sdw, bdw = f(inputs["wdw"]), f(inputs["sdw"]), f(inputs["bdw"])
    Wpw, spw, bpw = f(inputs["Wpw"]), f(inputs["spw"]), f(inputs["bpw"])
    Wr, sr, br = f(inputs["Wr"]), f(inputs["sr"]), f(inputs["br"])
    Wc, sc, bc = f(inputs["Wc"]), f(inputs["sc"]), f(inputs["bc"])
    Wp, sp, bp = f(inputs["Wp"]), f(inputs["sp"]), f(inputs["bp"])

    Wq_f = sq[:, None] * Wq
    Wk_f = sk[:, None] * Wk
    Wv_f = sv[:, None] * Wv

    g = wdw * sdw
    g_q, g_k, g_v = g[:NH_KD], g[NH_KD:2 * NH_KD], g[2 * NH_KD:]
    # z biases: q/k carry g in the conv weights; v's gate g is folded into
    # the pw weights and bv into the u eviction, so z_v bias is bdw/g.
    zb_q = g_q * bq + bdw[:NH_KD]
    zb_k = g_k * bk + bdw[NH_KD:2 * NH_KD]
    zb_v = bdw[2 * NH_KD:] / g_v
    zbias = np.concatenate([zb_q, zb_k, zb_v])          # 512

    def tiles2(a):   # (256, cols) -> [(128, cols)] * 2
        return [a[:128], a[128:]]

    wqkv = np.concatenate(
        tiles2((g_q[None, :] * Wq_f.T)) + tiles2((g_k[None, :] * Wk_f.T))
        + tiles2(Wv_f.T), axis=1)
    consts = {"wqkv": fb(wqkv)}
    # pw conv (bf16): /6 from h_sigmoid + g_v on the v input-channels
    Wpw_g = (spw[:, None] * Wpw) / 6.0
    Wpw_g = Wpw_g * np.concatenate([np.ones(256, np.float32), g_v])[None, :]
    # transposed tiles per input k-tile: [128, 256] = [m0 128 | m1 128]
    wpwb = np.concatenate([Wpw_g.T[128 * k:128 * (k + 1)] for k in range(4)],
                          axis=1)
    consts["wpwb"] = fb(wpwb)
    # padded head layout for the attention q/k weights (1/mean fold included)
    assert h == w, "mean folds assume H == W"
    wqTp = np.zeros((DIM, 256), np.float32)
    wkTp = np.zeros((DIM, 256), np.float32)
    qk_b = np.zeros((128, 512), np.float32)
    pe_rq = _interp_pos_np(f(inputs["pe_rq"]), h)
    pe_rk = _interp_pos_np(f(inputs["pe_rk"]), h)
    pe_cq = _interp_pos_np(f(inputs["pe_cq"]), w)
    pe_ck = _interp_pos_np(f(inputs["pe_ck"]), w)
    for hh in range(HEADS):
        sl_p = slice(32 * hh, 32 * hh + KEY_DIM)
        sl_c = slice(KEY_DIM * hh, KEY_DIM * (hh + 1))
        wqTp[:, sl_p] = (Wq_f[sl_c, :] / w).T
        wkTp[:, sl_p] = (Wk_f[sl_c, :] / w).T
        t, j = hh // 4, hh % 4
        prow = slice(32 * j, 32 * j + KEY_DIM)
        qk_b[prow, 128 * t:128 * t + 64] = bq[sl_c, None] + pe_rq[sl_c, :]
        qk_b[prow, 128 * t + 64:128 * t + 128] = bq[sl_c, None] + pe_cq[sl_c, :]
        qk_b[prow, 256 + 128 * t:256 + 128 * t + 64] = (
            bk[sl_c, None] + pe_rk[sl_c, :])
        qk_b[prow, 256 + 128 * t + 64:256 + 128 * t + 128] = (
            bk[sl_c, None] + pe_ck[sl_c, :])
    consts["qkbias"] = f(qk_b)
    id64pad = np.zeros((128, 64), np.float32)
    id64pad[:64] = np.eye(64, dtype=np.float32)
    wbigb = np.concatenate(
        tiles2((sr[:, None] * Wr).T) + tiles2((sc[:, None] * Wc).T)
        + tiles2(wqTp) + tiles2(wkTp) + tiles2(Wv_f.T)
        + [id64pad], axis=1)
    consts["wbigb"] = fb(wbigb)
    # fp8 DoubleRow final-conv weights: x16 (xx is divided by 16 on evict)
    Wp_f = sp[:, None] * Wp
    wp8 = np.zeros((128, 512), np.float32)
    for mo in range(2):
        for i in range(2):
            wp8[:, 256 * mo + 128 * i:256 * mo + 128 * (i + 1)] = (
                16.0 * Wp_f[128 * mo:128 * (mo + 1),
                            128 * i:128 * (i + 1)].T)
    consts["wp8"] = np.ascontiguousarray(wp8.astype(ml_dtypes.float8_e4m3))

    params = np.zeros((128, 20), np.float32)
    params[:, 2:6] = zbias.reshape(4, 128).T
    params[:, 6:8] = bv.reshape(2, 128).T
    params[:, 8:10] = br.reshape(2, 128).T
    params[:, 10:12] = bc.reshape(2, 128).T
    params[:, 12:14] = (bp + 3.0).reshape(2, 128).T
    params[:, 14:16] = (bpw / 6.0).reshape(2, 128).T
    consts["params"] = f(params)
    consts["onesW"] = np.full((max(h, w), 1), float(w),
                              ml_dtypes.bfloat16)
    consts["ones1"] = np.ones((1, 64), ml_dtypes.bfloat16)
    return consts


_NC_CACHE = {}


def _get_nc():
    if "nc" not in _NC_CACHE:
        _NC_CACHE["nc"] = build_nc()
    return _NC_CACHE["nc"]


def kernel(**inputs) -> np.ndarray:
    import ml_dtypes
    x = np.ascontiguousarray(
        np.asarray(inputs["x"], dtype=np.float32).astype(ml_dtypes.bfloat16))
    consts = prepare_consts(inputs)
    nc = _get_nc()
    in_maps = []
    for c in range(N_CORES):
        m = dict(consts)
        m["x"] = np.ascontiguousarray(
            x[c * BPC:(c + 1) * BPC].reshape(BPC, DIM, H * W))
        in_maps.append(m)
    res = bass_utils.run_bass_kernel_spmd(nc, in_maps,
                                          core_ids=list(range(N_CORES)))
    y = np.concatenate([np.asarray(r["y"], dtype=np.float32)
                        for r in res.results], axis=0)
    return y.reshape(B, DIM, H, W)
